# revision 6
# baseline (speedup 1.0000x reference)
"""GIN message-passing kernel (copy_u + segment_sum + residual) on 8 trn2 cores.

out = feat + segment_sum(feat[src], dst)   (N=100000, E=1600000, D=128)

Strategy (1D dst partition per the sharding hint, halo gather):
 - Each core owns a 12500-row shard of destination nodes and the edges whose
   dst falls in it. A self-loop per node folds the residual into the sum.
 - Host staging builds, per core and per supertile of 8 dst tiles, a local
   "halo table": the deduplicated source-feature rows referenced by that
   supertile's edges (plus a zeros row for slot padding), with edge indices
   renumbered into int16 local ids — the materialized halo exchange. Each
   table is its own DRAM tensor (dma_gather requires an offset-0 source).
 - Nodes in each shard are sorted by degree so each 128-node tile has
   near-uniform degree G_t (slot padding ~2%). Node p's messages occupy slot
   columns [0, G_t) of partition p.
 - Device, per tile: one dma_gather (single_packet=False — the single-packet
   mode caps an instruction at 64 descriptors per SDMA engine = 1024 idxs)
   pulls all 128*G_t message rows from the supertile's halo table into SBUF
   [128, G_t, 128]; one strided tensor_reduce sums the slot axis; one DMA
   writes the output tile.
 - Host unpermutes shard outputs and concatenates.
"""

import sys

if "/opt/trn_rl_repo" not in sys.path:
    sys.path.insert(0, "/opt/trn_rl_repo")

import numpy as np

N_NODES = 100000
N_EDGES = 1600000
D = 128
N_CORES = 8
SHARD = N_NODES // N_CORES          # 12500
P = 128
NT = (SHARD + P - 1) // P           # 98 tiles per core
PAD = NT * P                        # 12544
ST_TILES = 8                        # tiles per supertile (halo table unit)
N_ST = (NT + ST_TILES - 1) // ST_TILES
SPLIT_COLS = 64                     # max slot columns per dma_gather

_nc_cache = {}


def _gather_parts(g):
    """Split g slot columns into near-equal parts of <= SPLIT_COLS."""
    n = -(-g // SPLIT_COLS)
    base = g // n
    rem = g % n
    return [base + (1 if i < rem else 0) for i in range(n)]


def _build(G, Rst):
    """Build + compile the per-core program (identical across cores)."""
    import concourse.bacc as bacc
    import concourse.tile as tile
    from concourse import mybir

    nc = bacc.Bacc("TRN2", target_bir_lowering=False, debug=False,
                   num_devices=N_CORES)
    tab_d = [nc.dram_tensor(f"tab{s}", [int(Rst[s]), D], mybir.dt.float32,
                            kind="ExternalInput").ap()
             for s in range(N_ST)]
    IW = int(8 * sum(G))
    idx_d = nc.dram_tensor("idx", [P, IW], mybir.dt.int16,
                           kind="ExternalInput").ap()
    out_d = nc.dram_tensor("out", [PAD, D], mybir.dt.float32,
                           kind="ExternalOutput").ap()

    with tile.TileContext(nc) as tc:
        with tc.tile_pool(name="idxp", bufs=1) as idxp, \
             tc.tile_pool(name="msgs", bufs=4) as msgsp, \
             tc.tile_pool(name="accp", bufs=4) as accp:
            idx_t = idxp.tile([P, IW], mybir.dt.int16)
            nc.sync.dma_start(idx_t[:], idx_d[:])
            icol = 0
            for t in range(NT):
                g = int(G[t])
                st = t // ST_TILES
                msgs = msgsp.tile([P, g * D], mybir.dt.float32, tag="msgs")
                c0 = 0
                for gs in _gather_parts(g):
                    n_idx = P * gs
                    nc.gpsimd.dma_gather(
                        out_ap=msgs[:, c0 * D:(c0 + gs) * D].rearrange(
                            "p (g f) -> p g f", g=gs),
                        in_ap=tab_d[st][:],
                        idxs_ap=idx_t[:, icol:icol + 8 * gs],
                        num_idxs=n_idx,
                        num_idxs_reg=n_idx,
                        elem_size=D,
                        single_packet=False,
                    )
                    c0 += gs
                    icol += 8 * gs
                acc = accp.tile([P, D], mybir.dt.float32, tag="acc")
                nc.vector.tensor_reduce(
                    out=acc[:],
                    in_=msgs[:].rearrange("p (g f) -> p f g", g=g),
                    axis=mybir.AxisListType.X,
                    op=mybir.AluOpType.add)
                nc.sync.dma_start(out_d[t * P:(t + 1) * P, :], acc[:])
    nc.compile()
    return nc


def _host_prep(feat, src, dst):
    """Shard + degree-sort + build halo tables and int16 slot-index streams."""
    deg = np.bincount(dst, minlength=N_NODES)

    order = np.argsort(dst, kind="stable")
    dst_s = dst[order]
    src_s = src[order]
    starts = np.searchsorted(dst_s, np.arange(N_NODES))
    slot = np.arange(N_EDGES, dtype=np.int64) - starts[dst_s]

    # per-core degree-sort permutations and global per-tile slot widths
    perms = []
    Gcs = []
    for c in range(N_CORES):
        degp = deg[c * SHARD:(c + 1) * SHARD] + 1          # +1 self-loop
        perm = np.argsort(-degp, kind="stable")
        perms.append(perm)
        sd = np.concatenate([degp[perm], np.zeros(PAD - SHARD, np.int64)])
        Gcs.append(sd[::P])
    G = np.maximum(np.max(np.stack(Gcs), axis=0), 1)       # [NT]
    woff = np.concatenate([[0], np.cumsum(G)]).astype(np.int64)
    W = int(G.sum())

    # per-core slot grid [P, W] holding GLOBAL src row of every slot, -1 = pad
    slot_src = np.full((N_CORES, P, W), -1, np.int64)
    for c in range(N_CORES):
        base = c * SHARD
        rank = np.empty(SHARD, np.int64)
        rank[perms[c]] = np.arange(SHARD)
        a = np.searchsorted(dst_s, base)
        b = np.searchsorted(dst_s, base + SHARD)
        r = rank[dst_s[a:b] - base]
        slot_src[c, r & (P - 1), woff[r >> 7] + slot[a:b]] = src_s[a:b]
        rs = rank
        slot_src[c, rs & (P - 1), woff[rs >> 7] + deg[base:base + SHARD]] = (
            base + np.arange(SHARD))

    # halo tables per (core, supertile) + per-tile local slot ids
    tabs = [[] for _ in range(N_CORES)]     # per core/st: unique global rows
    locs = [[] for _ in range(N_CORES)]     # per core/tile: local idx [P, G_t]
    n_uniq = np.zeros((N_CORES, N_ST), np.int64)
    for c in range(N_CORES):
        for s in range(N_ST):
            t0, t1 = s * ST_TILES, min((s + 1) * ST_TILES, NT)
            blk = slot_src[c, :, woff[t0]:woff[t1]]
            valid = blk >= 0
            uniq, inv = np.unique(blk[valid], return_inverse=True)
            loc = np.full(blk.shape, len(uniq), np.int64)   # pad -> zeros row
            loc[valid] = inv
            n_uniq[c, s] = len(uniq) + 1
            tabs[c].append(uniq)
            w0 = 0
            for t in range(t0, t1):
                g = int(G[t])
                locs[c].append(loc[:, w0:w0 + g])
                w0 += g
    Rst = n_uniq.max(axis=0)                # uniform table shapes across cores
    assert Rst.max() <= 32767, Rst.max()

    tables = []                              # [N_ST] of [N_CORES, Rst[s], D]
    for s in range(N_ST):
        tb = np.zeros((N_CORES, int(Rst[s]), D), np.float32)
        for c in range(N_CORES):
            u = tabs[c][s]
            tb[c, :len(u)] = feat[u]
        tables.append(tb)

    # int16 idx streams: per (tile, gather-part) a block of 8*gs columns,
    # stream i = g*128+p wrapped into 16 partitions and replicated x8
    IW = int(8 * G.sum())
    big_idx = np.empty((N_CORES, P, IW), np.int16)
    for c in range(N_CORES):
        icol = 0
        for t in range(NT):
            g = int(G[t])
            c0 = 0
            for gs in _gather_parts(g):
                stream = locs[c][t][:, c0:c0 + gs].T.reshape(-1)  # p-fastest
                wrapped = stream.reshape(8 * gs, 16).T            # [16, 8*gs]
                big_idx[c, :, icol:icol + 8 * gs] = np.tile(wrapped, (8, 1))
                c0 += gs
                icol += 8 * gs
        assert icol == IW

    return tables, big_idx, perms, tuple(int(g) for g in G), tuple(int(r) for r in Rst)


LAST_RUN = None


def kernel(feat, src, dst):
    global LAST_RUN
    feat = np.ascontiguousarray(np.asarray(feat), dtype=np.float32)
    src = np.asarray(src).astype(np.int64)
    dst = np.asarray(dst).astype(np.int64)
    assert feat.shape == (N_NODES, D) and src.shape == (N_EDGES,)

    tables, big_idx, perms, G, Rst = _host_prep(feat, src, dst)

    key = (G, Rst)
    if key not in _nc_cache:
        _nc_cache[key] = _build(G, Rst)
    nc = _nc_cache[key]

    from concourse.bass_utils import run_bass_kernel_spmd

    in_maps = []
    for c in range(N_CORES):
        m = {f"tab{s}": tables[s][c] for s in range(N_ST)}
        m["idx"] = np.ascontiguousarray(big_idx[c])
        in_maps.append(m)
    res = run_bass_kernel_spmd(nc, in_maps, core_ids=list(range(N_CORES)))
    LAST_RUN = res

    out = np.empty((N_NODES, D), np.float32)
    for c in range(N_CORES):
        oc = np.asarray(res.results[c]["out"])
        out[c * SHARD:(c + 1) * SHARD][perms[c]] = oc[:SHARD]
    return out
